# revision 64
# baseline (speedup 1.0000x reference)
"""Trainium2 Bass kernel for nn_CWDiscriminator (per-class 3-layer MLP).

reference:
    x = inputs.transpose(0, 2, 1)            # (B, C, F)
    h = relu(einsum('bcf,cfg->bcg', x, W1) + b1)
    h = relu(einsum('bcf,cfg->bcg', h, W2) + b2)
    out = einsum('bcf,cf->bc', h, W3) + b3   # (B, C)

B=16384, F=256, C=19. Data-parallel over B across 8 NeuronCores
(B_loc = 2048 per core), 4 batch sections of 512 (one fp32 PSUM bank
of moving dim).  All matmuls are bf16 (fp32r pays a serialized 4-byte
weight load per matmul; fp8 DoubleRow fails the 2e-2 accuracy gate --
each e4m3 tensor alone contributes ~2.7e-2 rel-l2).

Per core, per section, a software pipeline over classes c:
  - L1(cc):  H1.T = W1[c].T @ X.T   (2 m-halves x 2 k-accum matmuls,
    N=512).  PSUM evicted with fused bias+ReLU to bf16: m0 on ACT
    (activation) and m1 on DVE (tensor_scalar) so both halves evict in
    parallel.
  - L2(cc-2): H2.T = W2[c].T @ H1.T, same split eviction except the
    m1 half is deferred one step: its PSUM completes at the very end
    of the step, and evicting it immediately would head-of-line-block
    the DVE FIFO ahead of the next h1-m1 eviction that L2 waits on.
    (h2 is only read by the section-end L3 burst, so the extra step of
    eviction latency is free.)  The 2-step L1->L2 lag guarantees h1 is
    fully evicted before L2 issues.
  - L3: one mega-burst per section (cc==21) of narrow (128x19)
    matmuls: k=0 accumulates into PSUM bank A rows 0:19 (PE column
    group q0), k=1 into bank B rows 32:51 (q32) -- the two column
    groups run concurrently, and batching all 19 classes into one
    burst pays the ~300ns full<->narrow array transition once per
    section instead of per class.  (q64/q96 return garbage on this
    silicon -- do not use.)  Host sums the two k-half partials.

Startup is HBM-bandwidth-bound: all bulk loads ride the sync queue in
class-consumption order (dma_start dispatch costs ~0.7us of engine
queue time, so nothing shares the ACT queue with evictions), first
chunks are 1-2 classes so L1(0) starts ~8.5us in, and each next
x-slab's 5MB DMA is gated on section progress via a dummy 1-element
write so it cannot starve the startup-critical chunks.

Output per core is 2 x (C, B_loc) fp32 partials; host sums, transposes
and adds b3.  Measured ~176us (baseline 199us), rel_l2 3.9e-3.
"""

import sys
import types

import numpy as np
import ml_dtypes

B, F, C = 16384, 256, 19
NCORES = 8
B_LOC = B // NCORES          # 2048
SECTIONS = [512, 512, 512, 512]  # batch columns per PSUM-accum round
assert sum(SECTIONS) == 2048

BF16 = ml_dtypes.bfloat16


# ---------------------------------------------------------------------------
# axon environment shims (NTFF profile hook + artifact upload stub) and the
# one-wait-per-instruction legalizer this walrus build requires.
# ---------------------------------------------------------------------------

def _setup_axon_env():
    if 'antenv.axon_hooks' not in sys.modules:
        mod = types.ModuleType('antenv.axon_hooks')
        mod._hook = None
        mod.set_axon_ntff_profile_hook = lambda h: setattr(mod, '_hook', h)
        mod.get_axon_ntff_profile_hook = lambda: mod._hook
        sys.modules['antenv.axon_hooks'] = mod
        try:
            import antenv
            antenv.axon_hooks = mod
        except ImportError:
            pass
        try:
            from trn_agent_boot.trn_boot import _ntff_profile_via_ctypes
            mod._hook = _ntff_profile_via_ctypes('/opt/axon/libaxon_pjrt.so')
        except Exception:
            pass
    import concourse.bass_utils as bu
    bu.upload_artifacts = lambda tmpdir: 'file://' + str(tmpdir)


def _legalize_waits(nc):
    """walrus accepts at most ONE sync wait per engine instruction (2 for
    EventSemaphore). Split extras onto preceding same-engine NoOps."""
    import concourse.mybir as mybir
    n_split = 0
    for fn in nc.m.functions:
        for bb in fn.blocks:
            insts = bb.instructions
            out = []
            for inst in insts:
                si = inst.sync_info
                ow = list(si.on_wait) if si is not None and si.on_wait else []
                cap = 2 if inst.opcode == "EventSemaphore" else 1
                if len(ow) > cap:
                    keep = ow[-cap:]
                    for k, w in enumerate(ow[:-cap]):
                        nop = mybir.InstNoOp(
                            name=f"{inst.name}-wsplit{k}",
                            engine=inst.engine,
                            ins=[],
                            outs=[],
                            sync_info=mybir.SyncInfo(on_wait=[w], on_update=[]),
                        )
                        out.append(nop)
                        n_split += 1
                    inst.sync_info = mybir.SyncInfo(
                        on_wait=keep,
                        on_update=list(si.on_update) if si.on_update else [],
                    )
                out.append(inst)
            insts[:] = out
    return n_split


# ---------------------------------------------------------------------------
# device program
# ---------------------------------------------------------------------------

_CACHE = {}
last_results = None  # BassKernelResults of the most recent run (for test.py)


def _build_program():
    from contextlib import ExitStack
    import concourse.bass as bass
    import concourse.mybir as mybir
    import concourse.tile as tile

    F32 = mybir.dt.float32
    B16 = mybir.dt.bfloat16

    nc = bass.Bass()

    # xt: host-pretransposed input, [p, c, k, b] = x[b, 128k+p, c], bf16
    xtd = nc.declare_dram_parameter("xtd", [128, C, 2, B_LOC], B16,
                                    isOutput=False)
    w1t = nc.declare_dram_parameter("w1t", [128, C, 2, 2, 128], B16,
                                    isOutput=False)
    w2t = nc.declare_dram_parameter("w2t", [128, C * 2 * 2 * 128], B16,
                                    isOutput=False)
    w3m = nc.declare_dram_parameter("w3m", [128, C * 2 * C], B16,
                                    isOutput=False)
    b1s = nc.declare_dram_parameter("b1s", [128, C, 2], F32, isOutput=False)
    b2s = nc.declare_dram_parameter("b2s", [128, C, 2], F32, isOutput=False)
    # out[k, c, b]: partial per k-half; host sums the two halves.
    out = nc.declare_dram_parameter("out", [2, C, B_LOC], F32, isOutput=True)

    NSEC = len(SECTIONS)
    SEC = SECTIONS[0]

    with ExitStack() as ctx:
        tc = ctx.enter_context(tile.TileContext(nc))

        consts = ctx.enter_context(tc.tile_pool(name="consts", bufs=1))
        xt_pool = ctx.enter_context(tc.tile_pool(name="xt", bufs=2))
        h1_pool = ctx.enter_context(tc.tile_pool(name="h1p", bufs=3))
        h2_pool = ctx.enter_context(tc.tile_pool(name="h2p", bufs=19))
        out_pool = ctx.enter_context(tc.tile_pool(name="outp", bufs=1))

        ps_l1 = ctx.enter_context(
            tc.tile_pool(name="ps_l1", bufs=3, space="PSUM"))
        ps_l2 = ctx.enter_context(
            tc.tile_pool(name="ps_l2", bufs=3, space="PSUM"))
        # L3 partial-sum banks: k=0 accumulates in col-group 0 (rows 0:19),
        # k=1 in col-group 1 (rows 32:51) of a second bank, so the two
        # narrow matmuls run concurrently on different PE column groups.
        ps_3a = ctx.enter_context(
            tc.tile_pool(name="ps_3a", bufs=1, space="PSUM"))
        ps_3b = ctx.enter_context(
            tc.tile_pool(name="ps_3b", bufs=1, space="PSUM"))

        # ---- X.T section slabs stream on the sync ring, self-paced by
        # the xt pool slots; everything else rides the scalar ring.
        # slab 0 loads immediately (split by class range so L1(c=0) can
        # start early); later slabs are emitted inside the previous
        # section's pipeline, gated on its progress, so their DMA doesn't
        # steal HBM bandwidth from the weight loads at startup.
        xts0 = xt_pool.tile([128, C, 2, SEC], B16, tag="xt")
        # Startup is DMA-latency-bound: the first L1 matmul needs only
        # w1[0:2] (0.26 MB) and x[0:2] (0.5 MB).  Stage tiny first chunks
        # and defer everything not needed in the first few classes so the
        # SDMA rails aren't clogged when the pipeline wants to start.
        nc.sync.dma_start(xts0[:, 0:1], xtd[:, 0:1, :, 0:SEC])
        slabs = [xts0]

        # Weight loads interleaved in class-consumption order: the class-c
        # pipeline needs w1[c] first, w2[c] two iterations later.
        w1sb = consts.tile([128, C, 2, 2, 128], B16)
        w2sb = consts.tile([128, C * 2 * 2 * 128], B16)
        w3sb = consts.tile([128, C * 2 * C], B16)
        b1sb = consts.tile([128, C, 2], F32)
        b2sb = consts.tile([128, C, 2], F32)
        NW2 = C * 2 * 2 * 128  # 9728
        W2C = NW2 // C  # w2 bytes-per-class stride in the flat view

        # All bulk loads ride the sync queue in class-need order; the
        # scalar queue stays free for ACT evictions (each dma_start
        # dispatch costs ~0.7us of engine-queue time).
        # The first few weight dispatches ride the scalar queue: it is
        # idle until the first ACT eviction (~12.8us), so dispatching
        # there runs in parallel with the x chunks on sync and cuts
        # ~3us of serialized ~0.7us-per-dma_start dispatch time.
        nc.scalar.dma_start(w1sb[:, 0:1], w1t[:, 0:1])
        nc.sync.dma_start(xts0[:, 1:2], xtd[:, 1:2, :, 0:SEC])
        nc.scalar.dma_start(w1sb[:, 1:2], w1t[:, 1:2])
        nc.sync.dma_start(xts0[:, 2:3], xtd[:, 2:3, :, 0:SEC])
        nc.scalar.dma_start(w1sb[:, 2:3], w1t[:, 2:3])
        nc.scalar.dma_start(b1sb[:], b1s[:])
        nc.sync.dma_start(xts0[:, 3:5], xtd[:, 3:5, :, 0:SEC])
        nc.scalar.dma_start(w1sb[:, 3:5], w1t[:, 3:5])
        nc.scalar.dma_start(b2sb[:], b2s[:])
        nc.scalar.dma_start(w2sb[:, 0:1 * W2C], w2t[:, 0:1 * W2C])
        nc.sync.dma_start(xts0[:, 5:7], xtd[:, 5:7, :, 0:SEC])
        nc.sync.dma_start(w1sb[:, 5:7], w1t[:, 5:7])
        nc.sync.dma_start(w2sb[:, 1 * W2C:5 * W2C], w2t[:, 1 * W2C:5 * W2C])
        nc.sync.dma_start(w3sb[:], w3m[:])
        nc.sync.dma_start(xts0[:, 7:13], xtd[:, 7:13, :, 0:SEC])
        nc.sync.dma_start(w1sb[:, 7:13], w1t[:, 7:13])
        nc.sync.dma_start(w2sb[:, 5 * W2C:11 * W2C], w2t[:, 5 * W2C:11 * W2C])
        nc.sync.dma_start(xts0[:, 13:C], xtd[:, 13:C, :, 0:SEC])
        nc.sync.dma_start(w1sb[:, 13:C], w1t[:, 13:C])
        nc.sync.dma_start(w2sb[:, 11 * W2C:], w2t[:, 11 * W2C:])

        w1v = w1sb[:]
        w2v = w2sb[:].rearrange("p (c k m j) -> p c k m j", c=C, k=2, m=2)
        w3v = w3sb[:].rearrange("p (c k q) -> p c k q", c=C, k=2)

        # Later slabs are allocated and DMA-started from inside the
        # previous section's class loop (gated on pipeline progress via
        # a dummy 1-element write) so their 5 MB transfers don't steal
        # HBM bandwidth from the startup-critical chunks.
        slabs.extend([None] * (NSEC - 1))

        def _emit_out_evict(po):
            ps3a_p, ps3b_p, sec_p = po
            out_sb = out_pool.tile([64, SEC], F32, tag="osb")
            nc.scalar.copy(out_sb[0:C], ps3a_p[0:C])
            nc.vector.tensor_copy(out_sb[32:32 + C], ps3b_p[32:32 + C])
            # Out DMA processes its 2KB row-packets serially on a single
            # SDMA engine (~80ns each); split each half across the sync
            # and scalar queues so four engines work in parallel.
            nc.sync.dma_start(out[0, 0:10, sec_p:sec_p + SEC],
                              out_sb[0:10])
            nc.scalar.dma_start(out[0, 10:C, sec_p:sec_p + SEC],
                                out_sb[10:C])
            nc.sync.dma_start(out[1, 0:10, sec_p:sec_p + SEC],
                              out_sb[32:42])
            nc.scalar.dma_start(out[1, 10:C, sec_p:sec_p + SEC],
                                out_sb[42:32 + C])

        for h in range(NSEC):
            xtv = slabs[h][:]
            sec0 = h * SEC
            ps3a = ps_3a.tile([128, SEC], mybir.dt.float32, tag="ps3a")
            ps3b = ps_3b.tile([128, SEC], mybir.dt.float32, tag="ps3b")
            h1_t = [None, None, None]
            h2_t = [None] * 19
            pend = None  # (psum, h2, c): h2 m1 eviction deferred a step
            # Step order on the PE queue: L1(cc), L3-burst, L2(cc-2).
            # L2 lags two steps so h1 is fully evicted well before its
            # matmuls issue.  L3 runs every 4th step as a burst over 4
            # classes (q0 matmuls chained, q32 riding concurrently on
            # the second PE column group / second PSUM bank), so the
            # ~300ns full<->narrow array transition is paid once per
            # four classes instead of once per class.
            for cc in range(C + 4):
                if cc < C:
                    c = cc
                    h1 = h1_pool.tile([128, 2, SEC], B16, tag="h1")
                    h1_t[c % 3] = h1
                    for m in range(2):
                        pg = ps_l1.tile([128, SEC], mybir.dt.float32,
                                        tag="pg1")
                        for k in range(2):
                            nc.tensor.matmul(
                                pg[:], w1v[:, c, k, m, :],
                                xtv[:, c, k, :],
                                start=(k == 0), stop=(k == 1))
                        if m == 0:
                            nc.scalar.activation(
                                h1[:, m, :], pg[:],
                                mybir.ActivationFunctionType.Relu,
                                bias=b1sb[:, c, m:m+1])
                        else:
                            nc.vector.tensor_scalar(
                                h1[:, m, :], pg[:],
                                b1sb[:, c, m:m+1], 0.0,
                                mybir.AluOpType.add, mybir.AluOpType.max)
                    if cc == 8 and h + 1 < NSEC:
                        xts = xt_pool.tile([128, C, 2, SEC], B16,
                                           tag="xt", name=f"xts{h+1}")
                        # dummy write from h1 delays the slab DMA until
                        # this section is well underway (WAW ordering).
                        nc.gpsimd.tensor_copy(
                            xts[0:1, 0:1, 0:1, 0:1], h1[0:1, 0:1, 0:1])
                        nc.sync.dma_start(
                            xts[:],
                            xtd[:, :, :, (h + 1) * SEC:(h + 2) * SEC])
                        slabs[h + 1] = xts
                # Flush last step's deferred h2-m1 eviction: its PSUM
                # completed at the very end of that step, so evicting it
                # here keeps DVE's strict FIFO from head-of-line blocking
                # the next h1-m1 eviction (which L2 waits on).  h2 isn't
                # consumed until the section-end L3 burst, so the extra
                # step of latency is free.
                if pend is not None:
                    pgp, h2p, cp = pend
                    nc.vector.tensor_scalar(
                        h2p[:, 1, :], pgp[:],
                        b2sb[:, cp, 1:2], 0.0,
                        mybir.AluOpType.add, mybir.AluOpType.max)
                    pend = None
                burst = {21: (0, 19)}
                if cc in burst:
                    for c in range(*burst[cc]):
                        h2 = h2_t[c % 19]
                        nc.tensor.matmul(
                            ps3a[0:C], w3v[:, c, 0, :], h2[:, 0, :],
                            start=(c == 0), stop=(c == C - 1),
                            tile_position=(0, 0))
                        nc.tensor.matmul(
                            ps3b[32:32 + C], w3v[:, c, 1, :],
                            h2[:, 1, :],
                            start=(c == 0), stop=(c == C - 1),
                            tile_position=(0, 32))
                if 2 <= cc <= C + 1:
                    c = cc - 2
                    h1 = h1_t[c % 3]
                    h2 = h2_pool.tile([128, 2, SEC], B16, tag="h2")
                    h2_t[c % 19] = h2
                    for m in range(2):
                        pg = ps_l2.tile([128, SEC], mybir.dt.float32,
                                        tag="pg2")
                        for k in range(2):
                            nc.tensor.matmul(
                                pg[:], w2v[:, c, k, m, :],
                                h1[:, k, :],
                                start=(k == 0), stop=(k == 1))
                        if m == 0:
                            nc.scalar.activation(
                                h2[:, m, :], pg[:],
                                mybir.ActivationFunctionType.Relu,
                                bias=b2sb[:, c, m:m+1])
                        else:
                            pend = (pg, h2, c)

            # Evict the L3 partials right here: during the mega-burst
            # ACT/DVE are idle, so the copies are free of contention.
            _emit_out_evict((ps3a, ps3b, sec0))

    _legalize_waits(nc)
    return nc


def _get_program():
    if 'nc' not in _CACHE:
        _setup_axon_env()
        _CACHE['nc'] = _build_program()
    return _CACHE['nc']


# ---------------------------------------------------------------------------
# host wrapper
# ---------------------------------------------------------------------------

def kernel(inputs, W1, b1, W2, b2, W3, b3):
    global last_results
    from concourse.bass_utils import run_bass_kernel_spmd

    nc = _get_program()

    inputs = np.asarray(inputs)
    W1 = np.asarray(W1, dtype=np.float32)
    b1 = np.asarray(b1, dtype=np.float32)
    W2 = np.asarray(W2, dtype=np.float32)
    b2 = np.asarray(b2, dtype=np.float32)
    W3 = np.asarray(W3, dtype=np.float32)
    b3 = np.asarray(b3, dtype=np.float32)

    # host-side layout prep for the shard: [p, c, k, b] = x[b, 128k+p, c]
    xbf = np.asarray(inputs).reshape(B, 2, 128, C).astype(BF16)
    xtd_full = np.ascontiguousarray(xbf.transpose(2, 3, 1, 0))

    # lhsT tiles: w1t[p, c, k, m, j] = W1[c, 128k+p, 128m+j]
    w1t = np.ascontiguousarray(
        W1.reshape(C, 2, 128, 2, 128).transpose(2, 0, 1, 3, 4)).astype(BF16)
    w2t = np.ascontiguousarray(
        W2.reshape(C, 2, 128, 2, 128).transpose(2, 0, 1, 3, 4)
    ).reshape(128, C * 2 * 2 * 128).astype(BF16)
    # w3m[p, c, k, c'] = (c'==c) * W3[c, 128k+p]
    w3m = np.zeros((128, C, 2, C), dtype=np.float32)
    for c in range(C):
        w3m[:, c, 0, c] = W3[c, :128]
        w3m[:, c, 1, c] = W3[c, 128:]
    w3m = w3m.reshape(128, C * 2 * C).astype(BF16)
    # b1s[p, c, m] = b1[c, 128m+p]
    b1s = np.ascontiguousarray(
        b1.reshape(C, 2, 128).transpose(2, 0, 1)).astype(np.float32)
    b2s = np.ascontiguousarray(
        b2.reshape(C, 2, 128).transpose(2, 0, 1)).astype(np.float32)

    core_ids = list(range(NCORES))
    in_maps = []
    for i in core_ids:
        in_maps.append({
            "xtd": np.ascontiguousarray(
                xtd_full[:, :, :, i * B_LOC:(i + 1) * B_LOC]),
            "w1t": w1t, "w2t": w2t, "w3m": w3m, "b1s": b1s, "b2s": b2s,
        })

    import os
    trace = bool(os.environ.get("BASS_TRACE"))
    res = run_bass_kernel_spmd(nc, in_maps, core_ids, trace=trace)
    last_results = res

    out_full = np.empty((B, C), dtype=np.float32)
    for i in core_ids:
        o2 = res.results[i]["out"]
        out_full[i * B_LOC:(i + 1) * B_LOC] = (o2[0] + o2[1]).T
    out_full += b3[None, :]
    return out_full



# revision 65
# speedup vs baseline: 1.0148x; 1.0148x over previous
"""Trainium2 Bass kernel for nn_CWDiscriminator (per-class 3-layer MLP).

reference:
    x = inputs.transpose(0, 2, 1)            # (B, C, F)
    h = relu(einsum('bcf,cfg->bcg', x, W1) + b1)
    h = relu(einsum('bcf,cfg->bcg', h, W2) + b2)
    out = einsum('bcf,cf->bc', h, W3) + b3   # (B, C)

B=16384, F=256, C=19. Data-parallel over B across 8 NeuronCores
(B_loc = 2048 per core), 4 batch sections of 512 (one fp32 PSUM bank
of moving dim).  All matmuls are bf16 (fp32r pays a serialized 4-byte
weight load per matmul; fp8 DoubleRow fails the 2e-2 accuracy gate --
each e4m3 tensor alone contributes ~2.7e-2 rel-l2).

Per core, per section, a software pipeline over classes c:
  - L1(cc):  H1.T = W1[c].T @ X.T   (2 m-halves x 2 k-accum matmuls,
    N=512).  PSUM evicted with fused bias+ReLU to bf16: m0 on ACT
    (activation) and m1 on DVE (tensor_scalar) so both halves evict in
    parallel.
  - L2(cc-2): H2.T = W2[c].T @ H1.T, same split eviction except the
    m1 half is deferred one step: its PSUM completes at the very end
    of the step, and evicting it immediately would head-of-line-block
    the DVE FIFO ahead of the next h1-m1 eviction that L2 waits on.
    (h2 is only read by the section-end L3 burst, so the extra step of
    eviction latency is free.)  The 2-step L1->L2 lag guarantees h1 is
    fully evicted before L2 issues.
  - L3: one mega-burst per section (cc==21) of narrow (128x19)
    matmuls: k=0 accumulates into PSUM bank A rows 0:19 (PE column
    group q0), k=1 into bank B rows 32:51 (q32) -- the two column
    groups run concurrently, and batching all 19 classes into one
    burst pays the ~300ns full<->narrow array transition once per
    section instead of per class.  (q64/q96 return garbage on this
    silicon -- do not use.)  Host sums the two k-half partials.

Startup is HBM-bandwidth-bound: all bulk loads ride the sync queue in
class-consumption order (dma_start dispatch costs ~0.7us of engine
queue time, so nothing shares the ACT queue with evictions), first
chunks are 1-2 classes so L1(0) starts ~8.5us in, and each next
x-slab's 5MB DMA is gated on section progress via a dummy 1-element
write so it cannot starve the startup-critical chunks.

Output per core is 2 x (C, B_loc) fp32 partials; host sums, transposes
and adds b3.  Measured ~176us (baseline 199us), rel_l2 3.9e-3.
"""

import sys
import types

import numpy as np
import ml_dtypes

B, F, C = 16384, 256, 19
NCORES = 8
B_LOC = B // NCORES          # 2048
SECTIONS = [512, 512, 512, 512]  # batch columns per PSUM-accum round
assert sum(SECTIONS) == 2048

BF16 = ml_dtypes.bfloat16


# ---------------------------------------------------------------------------
# axon environment shims (NTFF profile hook + artifact upload stub) and the
# one-wait-per-instruction legalizer this walrus build requires.
# ---------------------------------------------------------------------------

def _setup_axon_env():
    if 'antenv.axon_hooks' not in sys.modules:
        mod = types.ModuleType('antenv.axon_hooks')
        mod._hook = None
        mod.set_axon_ntff_profile_hook = lambda h: setattr(mod, '_hook', h)
        mod.get_axon_ntff_profile_hook = lambda: mod._hook
        sys.modules['antenv.axon_hooks'] = mod
        try:
            import antenv
            antenv.axon_hooks = mod
        except ImportError:
            pass
        try:
            from trn_agent_boot.trn_boot import _ntff_profile_via_ctypes
            mod._hook = _ntff_profile_via_ctypes('/opt/axon/libaxon_pjrt.so')
        except Exception:
            pass
    import concourse.bass_utils as bu
    bu.upload_artifacts = lambda tmpdir: 'file://' + str(tmpdir)


def _legalize_waits(nc):
    """walrus accepts at most ONE sync wait per engine instruction (2 for
    EventSemaphore). Split extras onto preceding same-engine NoOps."""
    import concourse.mybir as mybir
    n_split = 0
    for fn in nc.m.functions:
        for bb in fn.blocks:
            insts = bb.instructions
            out = []
            for inst in insts:
                si = inst.sync_info
                ow = list(si.on_wait) if si is not None and si.on_wait else []
                cap = 2 if inst.opcode == "EventSemaphore" else 1
                if len(ow) > cap:
                    keep = ow[-cap:]
                    for k, w in enumerate(ow[:-cap]):
                        nop = mybir.InstNoOp(
                            name=f"{inst.name}-wsplit{k}",
                            engine=inst.engine,
                            ins=[],
                            outs=[],
                            sync_info=mybir.SyncInfo(on_wait=[w], on_update=[]),
                        )
                        out.append(nop)
                        n_split += 1
                    inst.sync_info = mybir.SyncInfo(
                        on_wait=keep,
                        on_update=list(si.on_update) if si.on_update else [],
                    )
                out.append(inst)
            insts[:] = out
    return n_split


# ---------------------------------------------------------------------------
# device program
# ---------------------------------------------------------------------------

_CACHE = {}
last_results = None  # BassKernelResults of the most recent run (for test.py)


def _build_program():
    from contextlib import ExitStack
    import concourse.bass as bass
    import concourse.mybir as mybir
    import concourse.tile as tile

    F32 = mybir.dt.float32
    B16 = mybir.dt.bfloat16

    nc = bass.Bass()

    # xt: host-pretransposed input, [p, c, k, b] = x[b, 128k+p, c], bf16
    xtd = nc.declare_dram_parameter("xtd", [128, C, 2, B_LOC], B16,
                                    isOutput=False)
    w1t = nc.declare_dram_parameter("w1t", [128, C, 2, 2, 128], B16,
                                    isOutput=False)
    w2t = nc.declare_dram_parameter("w2t", [128, C * 2 * 2 * 128], B16,
                                    isOutput=False)
    w3m = nc.declare_dram_parameter("w3m", [128, C * 2 * C], B16,
                                    isOutput=False)
    b1s = nc.declare_dram_parameter("b1s", [128, C, 2], F32, isOutput=False)
    b2s = nc.declare_dram_parameter("b2s", [128, C, 2], F32, isOutput=False)
    # out[k, c, b]: partial per k-half; host sums the two halves.
    out = nc.declare_dram_parameter("out", [2, C, B_LOC], F32, isOutput=True)

    NSEC = len(SECTIONS)
    SEC = SECTIONS[0]

    with ExitStack() as ctx:
        tc = ctx.enter_context(tile.TileContext(nc))

        consts = ctx.enter_context(tc.tile_pool(name="consts", bufs=1))
        xt_pool = ctx.enter_context(tc.tile_pool(name="xt", bufs=2))
        h1_pool = ctx.enter_context(tc.tile_pool(name="h1p", bufs=3))
        h2_pool = ctx.enter_context(tc.tile_pool(name="h2p", bufs=19))
        out_pool = ctx.enter_context(tc.tile_pool(name="outp", bufs=1))

        ps_l1 = ctx.enter_context(
            tc.tile_pool(name="ps_l1", bufs=3, space="PSUM"))
        ps_l2 = ctx.enter_context(
            tc.tile_pool(name="ps_l2", bufs=3, space="PSUM"))
        # L3 partial-sum banks: k=0 accumulates in col-group 0 (rows 0:19),
        # k=1 in col-group 1 (rows 32:51) of a second bank, so the two
        # narrow matmuls run concurrently on different PE column groups.
        ps_3a = ctx.enter_context(
            tc.tile_pool(name="ps_3a", bufs=1, space="PSUM"))
        ps_3b = ctx.enter_context(
            tc.tile_pool(name="ps_3b", bufs=1, space="PSUM"))

        # ---- X.T section slabs stream on the sync ring, self-paced by
        # the xt pool slots; everything else rides the scalar ring.
        # slab 0 loads immediately (split by class range so L1(c=0) can
        # start early); later slabs are emitted inside the previous
        # section's pipeline, gated on its progress, so their DMA doesn't
        # steal HBM bandwidth from the weight loads at startup.
        xts0 = xt_pool.tile([128, C, 2, SEC], B16, tag="xt")
        # Startup is DMA-latency-bound: the first L1 matmul needs only
        # w1[0:2] (0.26 MB) and x[0:2] (0.5 MB).  Stage tiny first chunks
        # and defer everything not needed in the first few classes so the
        # SDMA rails aren't clogged when the pipeline wants to start.
        nc.sync.dma_start(xts0[:, 0:1], xtd[:, 0:1, :, 0:SEC])
        slabs = [xts0]

        # Weight loads interleaved in class-consumption order: the class-c
        # pipeline needs w1[c] first, w2[c] two iterations later.
        w1sb = consts.tile([128, C, 2, 2, 128], B16)
        w2sb = consts.tile([128, C * 2 * 2 * 128], B16)
        w3sb = consts.tile([128, C * 2 * C], B16)
        b1sb = consts.tile([128, C, 2], F32)
        b2sb = consts.tile([128, C, 2], F32)
        NW2 = C * 2 * 2 * 128  # 9728
        W2C = NW2 // C  # w2 bytes-per-class stride in the flat view

        # All bulk loads ride the sync queue in class-need order; the
        # scalar queue stays free for ACT evictions (each dma_start
        # dispatch costs ~0.7us of engine-queue time).
        # The first few weight dispatches ride the scalar queue: it is
        # idle until the first ACT eviction (~12.8us), so dispatching
        # there runs in parallel with the x chunks on sync and cuts
        # ~3us of serialized ~0.7us-per-dma_start dispatch time.
        nc.scalar.dma_start(w1sb[:, 0:1], w1t[:, 0:1])
        nc.sync.dma_start(xts0[:, 1:3], xtd[:, 1:3, :, 0:SEC])
        nc.scalar.dma_start(w1sb[:, 1:3], w1t[:, 1:3])
        nc.scalar.dma_start(b1sb[:], b1s[:])
        nc.sync.dma_start(xts0[:, 3:7], xtd[:, 3:7, :, 0:SEC])
        nc.scalar.dma_start(b2sb[:], b2s[:])
        nc.scalar.dma_start(w2sb[:, 0:1 * W2C], w2t[:, 0:1 * W2C])
        nc.sync.dma_start(w1sb[:, 3:7], w1t[:, 3:7])
        nc.sync.dma_start(w2sb[:, 1 * W2C:5 * W2C], w2t[:, 1 * W2C:5 * W2C])
        nc.sync.dma_start(w3sb[:], w3m[:])
        nc.sync.dma_start(xts0[:, 7:13], xtd[:, 7:13, :, 0:SEC])
        nc.sync.dma_start(w1sb[:, 7:13], w1t[:, 7:13])
        nc.sync.dma_start(w2sb[:, 5 * W2C:11 * W2C], w2t[:, 5 * W2C:11 * W2C])
        nc.sync.dma_start(xts0[:, 13:C], xtd[:, 13:C, :, 0:SEC])
        nc.sync.dma_start(w1sb[:, 13:C], w1t[:, 13:C])
        nc.sync.dma_start(w2sb[:, 11 * W2C:], w2t[:, 11 * W2C:])

        w1v = w1sb[:]
        w2v = w2sb[:].rearrange("p (c k m j) -> p c k m j", c=C, k=2, m=2)
        w3v = w3sb[:].rearrange("p (c k q) -> p c k q", c=C, k=2)

        # Later slabs are allocated and DMA-started from inside the
        # previous section's class loop (gated on pipeline progress via
        # a dummy 1-element write) so their 5 MB transfers don't steal
        # HBM bandwidth from the startup-critical chunks.
        slabs.extend([None] * (NSEC - 1))

        def _emit_out_evict(po):
            ps3a_p, ps3b_p, sec_p = po
            out_sb = out_pool.tile([64, SEC], F32, tag="osb")
            nc.scalar.copy(out_sb[0:C], ps3a_p[0:C])
            nc.vector.tensor_copy(out_sb[32:32 + C], ps3b_p[32:32 + C])
            # Out DMA processes its 2KB row-packets serially on a single
            # SDMA engine (~80ns each); split each half across the sync
            # and scalar queues so four engines work in parallel.
            nc.sync.dma_start(out[0, 0:10, sec_p:sec_p + SEC],
                              out_sb[0:10])
            nc.scalar.dma_start(out[0, 10:C, sec_p:sec_p + SEC],
                                out_sb[10:C])
            nc.sync.dma_start(out[1, 0:10, sec_p:sec_p + SEC],
                              out_sb[32:42])
            nc.scalar.dma_start(out[1, 10:C, sec_p:sec_p + SEC],
                                out_sb[42:32 + C])

        for h in range(NSEC):
            xtv = slabs[h][:]
            sec0 = h * SEC
            ps3a = ps_3a.tile([128, SEC], mybir.dt.float32, tag="ps3a")
            ps3b = ps_3b.tile([128, SEC], mybir.dt.float32, tag="ps3b")
            h1_t = [None, None, None]
            h2_t = [None] * 19
            pend = None  # (psum, h2, c): h2 m1 eviction deferred a step
            # Step order on the PE queue: L1(cc), L3-burst, L2(cc-2).
            # L2 lags two steps so h1 is fully evicted well before its
            # matmuls issue.  L3 runs every 4th step as a burst over 4
            # classes (q0 matmuls chained, q32 riding concurrently on
            # the second PE column group / second PSUM bank), so the
            # ~300ns full<->narrow array transition is paid once per
            # four classes instead of once per class.
            for cc in range(C + 4):
                if cc < C:
                    c = cc
                    h1 = h1_pool.tile([128, 2, SEC], B16, tag="h1")
                    h1_t[c % 3] = h1
                    for m in range(2):
                        pg = ps_l1.tile([128, SEC], mybir.dt.float32,
                                        tag="pg1")
                        for k in range(2):
                            nc.tensor.matmul(
                                pg[:], w1v[:, c, k, m, :],
                                xtv[:, c, k, :],
                                start=(k == 0), stop=(k == 1))
                        if m == 0:
                            nc.scalar.activation(
                                h1[:, m, :], pg[:],
                                mybir.ActivationFunctionType.Relu,
                                bias=b1sb[:, c, m:m+1])
                        else:
                            nc.vector.tensor_scalar(
                                h1[:, m, :], pg[:],
                                b1sb[:, c, m:m+1], 0.0,
                                mybir.AluOpType.add, mybir.AluOpType.max)
                    if cc == 8 and h + 1 < NSEC:
                        xts = xt_pool.tile([128, C, 2, SEC], B16,
                                           tag="xt", name=f"xts{h+1}")
                        # dummy write from h1 delays the slab DMA until
                        # this section is well underway (WAW ordering).
                        nc.gpsimd.tensor_copy(
                            xts[0:1, 0:1, 0:1, 0:1], h1[0:1, 0:1, 0:1])
                        nc.sync.dma_start(
                            xts[:],
                            xtd[:, :, :, (h + 1) * SEC:(h + 2) * SEC])
                        slabs[h + 1] = xts
                # Flush last step's deferred h2-m1 eviction: its PSUM
                # completed at the very end of that step, so evicting it
                # here keeps DVE's strict FIFO from head-of-line blocking
                # the next h1-m1 eviction (which L2 waits on).  h2 isn't
                # consumed until the section-end L3 burst, so the extra
                # step of latency is free.
                if pend is not None:
                    pgp, h2p, cp = pend
                    nc.vector.tensor_scalar(
                        h2p[:, 1, :], pgp[:],
                        b2sb[:, cp, 1:2], 0.0,
                        mybir.AluOpType.add, mybir.AluOpType.max)
                    pend = None
                burst = {21: (0, 19)}
                if cc in burst:
                    for c in range(*burst[cc]):
                        h2 = h2_t[c % 19]
                        nc.tensor.matmul(
                            ps3a[0:C], w3v[:, c, 0, :], h2[:, 0, :],
                            start=(c == 0), stop=(c == C - 1),
                            tile_position=(0, 0))
                        nc.tensor.matmul(
                            ps3b[32:32 + C], w3v[:, c, 1, :],
                            h2[:, 1, :],
                            start=(c == 0), stop=(c == C - 1),
                            tile_position=(0, 32))
                if 2 <= cc <= C + 1:
                    c = cc - 2
                    h1 = h1_t[c % 3]
                    h2 = h2_pool.tile([128, 2, SEC], B16, tag="h2")
                    h2_t[c % 19] = h2
                    for m in range(2):
                        pg = ps_l2.tile([128, SEC], mybir.dt.float32,
                                        tag="pg2")
                        for k in range(2):
                            nc.tensor.matmul(
                                pg[:], w2v[:, c, k, m, :],
                                h1[:, k, :],
                                start=(k == 0), stop=(k == 1))
                        if m == 0:
                            nc.scalar.activation(
                                h2[:, m, :], pg[:],
                                mybir.ActivationFunctionType.Relu,
                                bias=b2sb[:, c, m:m+1])
                        else:
                            pend = (pg, h2, c)

            # Evict the L3 partials right here: during the mega-burst
            # ACT/DVE are idle, so the copies are free of contention.
            _emit_out_evict((ps3a, ps3b, sec0))

    _legalize_waits(nc)
    return nc


def _get_program():
    if 'nc' not in _CACHE:
        _setup_axon_env()
        _CACHE['nc'] = _build_program()
    return _CACHE['nc']


# ---------------------------------------------------------------------------
# host wrapper
# ---------------------------------------------------------------------------

def kernel(inputs, W1, b1, W2, b2, W3, b3):
    global last_results
    from concourse.bass_utils import run_bass_kernel_spmd

    nc = _get_program()

    inputs = np.asarray(inputs)
    W1 = np.asarray(W1, dtype=np.float32)
    b1 = np.asarray(b1, dtype=np.float32)
    W2 = np.asarray(W2, dtype=np.float32)
    b2 = np.asarray(b2, dtype=np.float32)
    W3 = np.asarray(W3, dtype=np.float32)
    b3 = np.asarray(b3, dtype=np.float32)

    # host-side layout prep for the shard: [p, c, k, b] = x[b, 128k+p, c]
    xbf = np.asarray(inputs).reshape(B, 2, 128, C).astype(BF16)
    xtd_full = np.ascontiguousarray(xbf.transpose(2, 3, 1, 0))

    # lhsT tiles: w1t[p, c, k, m, j] = W1[c, 128k+p, 128m+j]
    w1t = np.ascontiguousarray(
        W1.reshape(C, 2, 128, 2, 128).transpose(2, 0, 1, 3, 4)).astype(BF16)
    w2t = np.ascontiguousarray(
        W2.reshape(C, 2, 128, 2, 128).transpose(2, 0, 1, 3, 4)
    ).reshape(128, C * 2 * 2 * 128).astype(BF16)
    # w3m[p, c, k, c'] = (c'==c) * W3[c, 128k+p]
    w3m = np.zeros((128, C, 2, C), dtype=np.float32)
    for c in range(C):
        w3m[:, c, 0, c] = W3[c, :128]
        w3m[:, c, 1, c] = W3[c, 128:]
    w3m = w3m.reshape(128, C * 2 * C).astype(BF16)
    # b1s[p, c, m] = b1[c, 128m+p]
    b1s = np.ascontiguousarray(
        b1.reshape(C, 2, 128).transpose(2, 0, 1)).astype(np.float32)
    b2s = np.ascontiguousarray(
        b2.reshape(C, 2, 128).transpose(2, 0, 1)).astype(np.float32)

    core_ids = list(range(NCORES))
    in_maps = []
    for i in core_ids:
        in_maps.append({
            "xtd": np.ascontiguousarray(
                xtd_full[:, :, :, i * B_LOC:(i + 1) * B_LOC]),
            "w1t": w1t, "w2t": w2t, "w3m": w3m, "b1s": b1s, "b2s": b2s,
        })

    import os
    trace = bool(os.environ.get("BASS_TRACE"))
    res = run_bass_kernel_spmd(nc, in_maps, core_ids, trace=trace)
    last_results = res

    out_full = np.empty((B, C), dtype=np.float32)
    for i in core_ids:
        o2 = res.results[i]["out"]
        out_full[i * B_LOC:(i + 1) * B_LOC] = (o2[0] + o2[1]).T
    out_full += b3[None, :]
    return out_full

